# revision 1
# baseline (speedup 1.0000x reference)
"""Distributed HGNN+ convolution for 8 Trainium2 NeuronCores (Bass/Tile).

Math (dense hypergraph incidence H [N_V, N_E], features X [N_V, C]):
    Xt  = X @ W.T + b                    # theta
    Xe  = (H.T @ Xt) * 1/colsum(H)       # V2E mean aggregation
    Xv  = (H @ Xe)   * 1/rowsum(H)       # E2V mean aggregation
    out = relu(Xv)

Distribution (chunked-AllGather scheme): theta is vertex-row-parallel;
the V2E GEMM is EDGE-sharded (each core contracts over ALL vertices
for its 1/8 of the edges -- exact, no AllReduce needed) after an
AllGather of Xt; the E2V GEMM is vertex-row-parallel after a second
AllGather of Xe.  Cheap AllGathers replace the baseline's 8.4 MB-per-
rank AllReduce, and both passes keep identical per-core matmul cycles.

Overlap structure: AG1 is split in four (4 vertex tiles per core
each), and V2E runs in four phases -- ALL eight edge tiles contract
the phase-q vertices (8 concurrent PSUM banks) before phase q+1 -- so
after the first chunk lands, each ~55 us compute phase hides the next
~22 us gather chunk entirely.  AG2 is split in four (edge-tile pairs)
so the first three chunks fly while V2E still computes; E2V consumes
edge chunks in arrival order, late chunk last.  The vertex/edge
permutations implied by the chunked gathers are baked into the host-
side pack; device loops are oblivious to them.

The mean-aggregation degree scalings are folded into the host-side
pack of H (V2E panels column-scaled by 1/colsum, E2V panels row-scaled
by 1/rowsum), so every matmul is exactly N=512 (one PSUM bank, one
matmul per contraction tile) and every DMA chunk is 32B-aligned.

Compute is bf16 with fp32 PSUM accumulation; AllGathers move bf16.
"""

import contextlib

import numpy as np
import ml_dtypes

BF16 = ml_dtypes.bfloat16

# Problem shape (hardcoded per contract).
N_V, N_E, CH, NCORES = 16384, 8192, 512, 8
AG1_CHUNKS = 8
AG2_CHUNKS = 4


def _full_cfg():
    return dict(n_v=N_V, n_e=N_E, ch=CH, ncores=NCORES)


def _ag(nc, mybir, cfg, rg, src, dst):
    """AllGather, or a bandwidth-equivalent local DRAM copy for ablation."""
    if cfg.get("no_ag"):
        nc.sync.dma_start(dst[0 : src.shape[0], :], src)
    else:
        nc.gpsimd.collective_compute(
            "AllGather",
            mybir.AluOpType.bypass,
            replica_groups=rg,
            ins=[src.opt()],
            outs=[dst.opt()],
        )


def build_graph(tc, io, cfg):
    """Emit the Tile IR. io: dict of DRAM APs: hsp, htp, xta, wtb, out."""
    from concourse import mybir

    nc = tc.nc
    f32 = mybir.dt.float32
    bf16 = mybir.dt.bfloat16
    Relu = mybir.ActivationFunctionType.Relu

    n_v, n_e, ch, ncores = cfg["n_v"], cfg["n_e"], cfg["ch"], cfg["ncores"]
    VS = n_v // ncores      # vertices per core (2048)
    KV = VS // 128          # local vertex 128-tiles (16)
    NC1 = AG1_CHUNKS
    KVC = KV // NC1         # per-core vertex tiles per AG1 chunk (4)
    KVG = n_v // 128        # global vertex 128-tiles (128)
    ES = n_e // ncores      # edges per core (1024)
    EML = ES // 128         # local edge 128-tiles (8)
    EM = n_e // 128         # global edge 128-tiles (64)
    CK = ch // 128          # theta contraction tiles (4)
    CKT = CK + 1            # + the ones/bias rank-1 tile
    NPH = KVG // NC1        # vertex tiles per V2E phase/panel (32)
    HH = NPH * 128          # panel width (4096)
    NC2 = AG2_CHUNKS
    EMC = EML // NC2        # local edge tiles per AG2 chunk (2)
    rg = [list(range(ncores))]

    hsp, htp, xta, wtb, out = io["hsp"], io["htp"], io["xta"], io["wtb"], io["out"]

    with contextlib.ExitStack() as ctx:
        work_pool = ctx.enter_context(tc.tile_pool(name="work", bufs=1))
        panel = ctx.enter_context(tc.tile_pool(name="panel", bufs=3))
        htpool = ctx.enter_context(tc.tile_pool(name="htpool", bufs=3))
        sb_out = ctx.enter_context(tc.tile_pool(name="sb_out", bufs=3))
        psum = ctx.enter_context(tc.tile_pool(name="psum", bufs=8, space="PSUM"))
        dram = ctx.enter_context(tc.tile_pool(name="dram", bufs=1, space="DRAM"))

        # One big SBUF scratch (128 KB/partition); phases alias sub-ranges.
        # Tile's range-exact access tracking orders the reuse.
        work = work_pool.tile([128, KVG * ch], bf16)
        xt_all = work[:, 0 : KV * ch]                     # theta output
        xta_sb = work[:, KV * ch : KV * ch + CKT * VS]    # theta lhsT panels
        wof = KV * ch + CKT * VS
        wtb_sb = work[:, wof : wof + CKT * ch]            # theta rhs (W.T;b)
        xt_sb = work                                      # gathered Xt (phase B)
        xe_sb = work[:, 0 : EM * ch]                      # gathered Xe (phase C)

        ag1_in = [dram.tile([128, KVC * ch], bf16, name=f"ag1i{h}", tag=f"ag1i{h}")
                  for h in range(NC1)]
        ag1_out = [dram.tile([128 * ncores, KVC * ch], bf16, name=f"ag1o{h}",
                             tag=f"ag1o{h}", addr_space="Shared")
                   for h in range(NC1)]
        ag2_in = [dram.tile([128, EMC * ch], bf16, name=f"ag2i{k}", tag=f"ag2i{k}")
                  for k in range(NC2)]
        ag2_out = [dram.tile([128 * ncores, EMC * ch], bf16, name=f"ag2o{k}",
                             tag=f"ag2o{k}", addr_space="Shared")
                   for k in range(NC2)]

        # ---- theta: Xt = [X | 1] @ [W.T ; b], zero-padded to CKT full
        # 128-row contraction tiles (bias rides as rank-1 update in tile CK).
        # After each half of the vm tiles, its AG1 chunk is kicked off.
        if not cfg.get("skip_theta"):
            xta_v = xta_sb.rearrange("p (k f) -> p k f", k=CKT)
            xta_s = xta.rearrange("(k p) f -> p k f", p=128)
            for q in range(NC1):
                qs, qe = q * KVC * 128, (q + 1) * KVC * 128
                nc.sync.dma_start(xta_v[:, :, qs:qe], xta_s[:, :, qs:qe])
            nc.sync.dma_start(
                wtb_sb.rearrange("p (k f) -> p k f", k=CKT),
                wtb.rearrange("(k p) f -> p k f", p=128),
            )
            for vm in range(KV):
                ps = psum.tile([128, ch], f32, tag="ps", name="ps_theta")
                for kt in range(CKT):
                    nc.tensor.matmul(
                        ps,
                        lhsT=xta_sb[:, kt * VS + vm * 128 : kt * VS + (vm + 1) * 128],
                        rhs=wtb_sb[:, kt * ch : (kt + 1) * ch],
                        start=(kt == 0),
                        stop=(kt == CKT - 1),
                    )
                nc.vector.tensor_copy(xt_all[:, vm * ch : (vm + 1) * ch], ps)
                if (vm + 1) % KVC == 0:
                    q = vm // KVC
                    nc.sync.dma_start(
                        ag1_in[q], xt_all[:, q * KVC * ch : (q + 1) * KVC * ch])

        # ---- AllGather Xt in NC1 chunks; stream each back into SBUF in
        # rank-blocks.  xt_sb column order is the gather order: position
        # h*NPH + c*KVC + j holds core c's vertex tile h*KVC+j -- hsp
        # panels are packed to match.
        for h in range(NC1):
            _ag(nc, mybir, cfg, rg, ag1_in[h], ag1_out[h])
            for c in range(ncores):
                nc.sync.dma_start(
                    xt_sb[:, (h * NPH + c * KVC) * ch : (h * NPH + (c + 1) * KVC) * ch],
                    ag1_out[h][c * 128 : (c + 1) * 128, :],
                )

        # ---- V2E (edge-sharded): Xe[e_c] = (H*D_e^-1)[:, e_c].T @ Xt.
        # NC1 phases over the vertex chunks: ALL edge tiles contract
        # phase-q vertices (8 concurrent PSUM banks) before phase q+1,
        # so each phase's compute overlaps the next AG1 chunk.  Panels
        # stream 1 MB each, triple-buffered.
        if not cfg.get("skip_v2e"):
            ps_e = [psum.tile([128, ch], f32, tag="ps", name=f"psE{em}")
                    for em in range(EML)]

            def v2e_seg(em, half):
                hs_sb = panel.tile([128, HH], bf16, tag="panel", name="hs_sb")
                nc.sync.dma_start(hs_sb, hsp[half * EML + em])
                for u in range(NPH):
                    vk = half * NPH + u
                    nc.tensor.matmul(
                        ps_e[em],
                        lhsT=hs_sb[:, u * 128 : (u + 1) * 128],
                        rhs=xt_sb[:, vk * ch : (vk + 1) * ch],
                        start=(vk == 0),
                        stop=(vk == KVG - 1),
                    )

            # Early phases phase-major: every edge tile advances one vertex
            # chunk, so compute always has a gathered chunk to chew while
            # the next AG1 chunk is on the wire.
            for half in range(NC1 - 2):
                for em in range(EML):
                    v2e_seg(em, half)
            # Last two phases em-major: each edge tile completes its full
            # contraction and drains immediately, spreading the AG2 chunk
            # launches ~2 drains apart instead of bunching them inside one
            # short final phase (which would backlog the collective queue).
            for em in range(EML):
                for half in (NC1 - 2, NC1 - 1):
                    v2e_seg(em, half)
                ar_sb = sb_out.tile([128, ch], bf16, tag="ar_sb", name="ar_sb")
                nc.vector.tensor_copy(ar_sb, ps_e[em])
                k, j = divmod(em, EMC)
                nc.sync.dma_start(ag2_in[k][:, j * ch : (j + 1) * ch], ar_sb)
                if j == EMC - 1:
                    _ag(nc, mybir, cfg, rg, ag2_in[k], ag2_out[k])
                    for c in range(ncores):
                        nc.sync.dma_start(
                            xe_sb[:, (k * EMC * ncores + c * EMC) * ch
                                  : (k * EMC * ncores + (c + 1) * EMC) * ch],
                            ag2_out[k][c * 128 : (c + 1) * 128, :],
                        )
        elif not cfg.get("no_ag"):
            for k in range(NC2):
                _ag(nc, mybir, cfg, rg, ag2_in[k], ag2_out[k])

        # ---- E2V (row-parallel): Xv[v_c] = (D_v^-1 H)[v_c, :] @ Xe, ReLU.
        # xe_sb edge order is chunk-major gather order; htp is packed to
        # match, so the late chunks are consumed last.
        KE_EARLY = (NC2 - 1) * EMC * ncores   # positions before the last chunk

        def e2v_mm(ps, ht_sb, ke):
            nc.tensor.matmul(
                ps,
                lhsT=ht_sb[:, ke * 128 : (ke + 1) * 128],
                rhs=xe_sb[:, ke * ch : (ke + 1) * ch],
                start=(ke == 0),
                stop=(ke == EM - 1),
            )

        def e2v_tail(vm, ps):
            o_sb = sb_out.tile([128, ch], f32, tag="o_sb", name="o_sb")
            nc.scalar.activation(o_sb, ps, Relu)
            nc.sync.dma_start(out[vm * 128 : (vm + 1) * 128, :], o_sb)

        # The first two row-tiles run their early-chunk contractions before
        # either touches the last AG2 chunk: the in-order PE then has
        # ~2x10 us of ready work covering the final gather's ~16 us
        # latency, instead of stalling once at vm0's late positions.
        pend = None
        for vm in range(KV if not cfg.get("skip_e2v") else 0):
            ht_sb = htpool.tile([128, n_e], bf16, tag="ht", name="ht_sb")
            nc.sync.dma_start(ht_sb, htp[vm])
            ps = psum.tile([128, ch], f32, tag="ps", name="psV")
            if vm == 0:
                for ke in range(KE_EARLY):
                    e2v_mm(ps, ht_sb, ke)
                pend = (ps, ht_sb)
            elif vm == 1:
                for ke in range(KE_EARLY):
                    e2v_mm(ps, ht_sb, ke)
                ps0, ht0 = pend
                for ke in range(KE_EARLY, EM):
                    e2v_mm(ps0, ht0, ke)
                e2v_tail(0, ps0)
                for ke in range(KE_EARLY, EM):
                    e2v_mm(ps, ht_sb, ke)
                e2v_tail(1, ps)
            else:
                for ke in range(EM):
                    e2v_mm(ps, ht_sb, ke)
                e2v_tail(vm, ps)


def pack_inputs(X, H, W, b, cfg):
    """Host-side shard/scale/cast/pack. Returns one input map per core.

    The V2E/E2V mean-normalizations are applied here: hsp carries
    H * 1/colsum (column-scaled, natural layout, lhsT = vertex-major),
    htp carries H * 1/rowsum (row-scaled, transposed panels).  Vertex
    tiles are permuted into AG1-chunk gather order (core-major within
    each half); edge tiles into AG2-chunk gather order (chunk-major).
    """
    from concurrent.futures import ThreadPoolExecutor

    n_v, n_e, ch, ncores = cfg["n_v"], cfg["n_e"], cfg["ch"], cfg["ncores"]
    VS = n_v // ncores
    KV = VS // 128
    NC1 = AG1_CHUNKS
    KVC = KV // NC1
    KVG = n_v // 128
    ES = n_e // ncores
    EML = ES // 128
    EM = n_e // 128
    NPH = KVG // NC1
    NC2 = AG2_CHUNKS
    EMC = EML // NC2

    # Gather-order permutations (position -> global tile index).
    perm_v = [c * KV + h * KVC + j
              for h in range(NC1) for c in range(ncores) for j in range(KVC)]
    perm_e = [c * EML + k * EMC + j
              for k in range(NC2) for c in range(ncores) for j in range(EMC)]

    colsum = H.sum(axis=0, dtype=np.float64)
    rowsum = H.sum(axis=1, dtype=np.float64)
    rc = np.where(colsum == 0, 0.0, 1.0 / colsum).astype(np.float32)
    rr = np.where(rowsum == 0, 0.0, 1.0 / rowsum).astype(np.float32)

    wtb = np.vstack(
        [
            np.ascontiguousarray(W.T).astype(np.float32),
            b[None, :].astype(np.float32),
            np.zeros((127, ch), np.float32),
        ]
    ).astype(BF16)

    def pack_core(c):
        # V2E lhsT panels: column slice of H, column-scaled, vertex-major,
        # vertex tiles in perm_v order.
        # hsp[half*EML+em][p, u*128+f] = Hs[perm_v[half*NH+u]*128+p, em*128+f]
        Hc = (H[:, c * ES : (c + 1) * ES] * rc[None, c * ES : (c + 1) * ES]
              ).astype(BF16)
        Rv = Hc.reshape(KVG, 128, EML, 128)[perm_v]   # [vkpos, p, em, f]
        Rv = Rv.reshape(NC1, NPH, 128, EML, 128)
        hsp = np.ascontiguousarray(Rv.transpose(0, 3, 2, 1, 4)).reshape(
            NC1 * EML, 128, NPH * 128)
        # E2V lhsT panels: row slice of H, row-scaled, transposed, edge
        # tiles in perm_e order.
        # htp[vm][p, q*128+f] = Hr[vm*128+f, perm_e[q]*128+p]
        Hr = (H[c * VS : (c + 1) * VS, :] * rr[c * VS : (c + 1) * VS, None]
              ).astype(BF16)
        R2 = Hr.reshape(KV, 128, EM, 128).transpose(0, 3, 2, 1)  # [vm,pe,ke,fv]
        htp = np.ascontiguousarray(R2[:, :, perm_e, :]).reshape(KV, 128, n_e)
        Xc = X[c * VS : (c + 1) * VS]
        xta = np.vstack(
            [
                np.ascontiguousarray(Xc.T),
                np.ones((1, VS), np.float32),
                np.zeros((127, VS), np.float32),
            ]
        ).astype(BF16)
        return dict(hsp=hsp, htp=htp, xta=xta, wtb=wtb)

    with ThreadPoolExecutor(max_workers=ncores) as ex:
        return list(ex.map(pack_core, range(ncores)))


_cache = {}


def _build_compiled(cfg, reps=1):
    key = (tuple(sorted(cfg.items())), reps)
    if key in _cache:
        return _cache[key]
    from concourse import bacc, mybir, tile

    n_v, n_e, ch, ncores = cfg["n_v"], cfg["n_e"], cfg["ch"], cfg["ncores"]
    VS = n_v // ncores
    KV = VS // 128
    ES = n_e // ncores
    EML = ES // 128

    nc = bacc.Bacc("TRN2", target_bir_lowering=False, debug=False,
                   num_devices=ncores)
    io = {
        "hsp": nc.dram_tensor("hsp", [EML * AG1_CHUNKS, 128, (n_v // 128 // AG1_CHUNKS) * 128],
                              mybir.dt.bfloat16, kind="ExternalInput").ap(),
        "htp": nc.dram_tensor("htp", [KV, 128, n_e], mybir.dt.bfloat16,
                              kind="ExternalInput").ap(),
        "xta": nc.dram_tensor("xta", [ch + 128, VS], mybir.dt.bfloat16,
                              kind="ExternalInput").ap(),
        "wtb": nc.dram_tensor("wtb", [ch + 128, ch], mybir.dt.bfloat16,
                              kind="ExternalInput").ap(),
        "out": nc.dram_tensor("out", [VS, ch], mybir.dt.float32,
                              kind="ExternalOutput").ap(),
    }
    with tile.TileContext(nc) as tc:
        for _ in range(reps):
            build_graph(tc, io, cfg)
    nc.compile()
    _cache[key] = nc
    return nc


def kernel(X, H, W, b, _trace=False, _cfg=None, _reps=1):
    from concourse.bass_utils import run_bass_kernel_spmd

    cfg = _cfg or _full_cfg()
    X = np.asarray(X, dtype=np.float32)
    H = np.asarray(H, dtype=np.float32)
    W = np.asarray(W, dtype=np.float32)
    b = np.asarray(b, dtype=np.float32)

    nc = _build_compiled(cfg, reps=_reps)
    in_maps = pack_inputs(X, H, W, b, cfg)
    res = run_bass_kernel_spmd(
        nc, in_maps, core_ids=list(range(cfg["ncores"])), trace=_trace
    )
    kernel.last_result = res
    return np.concatenate([r["out"] for r in res.results], axis=0)


kernel.last_result = None

